# revision 35
# baseline (speedup 1.0000x reference)
"""GridTransformerBlock TRN2 kernel (v2).

Sharding: batch-parallel over B=8 -> one batch per NeuronCore, zero
collectives.

v2 design (vs v1 baseline at 1.147ms):
- bf16 operands everywhere (weights + activations). At N>=256 the PE streams
  1 row/cycle for both f32r and bf16, but bf16 gets Fast Weight Load
  (f32r LDWEIGHTS was 226ns x 3584 = 810us of PE weight-path time), 1.0
  (vs 1.5) cycles/row transposes, and halves SBUF/DMA traffic.
- Phase split: ALL window attention first (ACT table = exp only), then ALL
  FFN (ACT table = gelu only). v1 alternated per stripe and paid ~44
  ACT_TABLE_LOADs. ys (8.4MB bf16) stays SBUF-resident between phases.
- Window-pair batching: Q/K projections stream N=512 through one weight
  load; software-pipelined slot schedule keeps the PE fed across the
  transpose->copy->matmul dependency chains (v1 had 590us of HAM throttle
  from PE idle gaps).
- FFN W2 matmul emits token-major output directly (lhsT = gelu chunks),
  killing the f^T transposes + copies; LN epilogue fused into
  tensor_scalar ops; 2-iteration Newton rsqrt.
"""

import os
import sys
import numpy as np

for _p in ("/opt/trn_rl_repo", "/root/.axon_site/_ro/trn_rl_repo"):
    if _p not in sys.path and os.path.isdir(_p):
        sys.path.insert(0, _p)

B, S, E, FF = 8, 16384, 256, 1024
H, W, G = 128, 128, 16
Hn, Wn = 8, 8
NWP = Hn * (Wn // 2)   # 32 window-pairs
NNB = 32               # 32 FFN blocks of 512 tokens

_CACHE = {}


def _build(flags):
    use_bqk, use_bv, use_bo, use_b1, use_b2, use_g1, use_g2 = flags
    import concourse.bacc as bacc
    import concourse.mybir as mybir
    import concourse.tile as tile
    from contextlib import ExitStack

    F32 = mybir.dt.float32
    BF16 = mybir.dt.bfloat16
    I32 = mybir.dt.int32
    AF = mybir.ActivationFunctionType
    OP = mybir.AluOpType

    nc = bacc.Bacc("TRN2", target_bir_lowering=False, debug=False, num_devices=8)

    x_d = nc.dram_tensor("x", [S, E], BF16, kind="ExternalInput")
    wq_d = nc.dram_tensor("wq", [E, E], BF16, kind="ExternalInput")
    wk_d = nc.dram_tensor("wk", [E, E], BF16, kind="ExternalInput")
    wv_d = nc.dram_tensor("wv", [E, E], BF16, kind="ExternalInput")
    wo_d = nc.dram_tensor("wo", [E, E], BF16, kind="ExternalInput")
    w1_d = nc.dram_tensor("w1", [E, FF], BF16, kind="ExternalInput")
    w2_d = nc.dram_tensor("w2", [FF, E], BF16, kind="ExternalInput")
    id_d = nc.dram_tensor("ident", [128, 128], BF16, kind="ExternalInput")
    out_d = nc.dram_tensor("out", [S, E], F32, kind="ExternalOutput")
    if use_bqk:
        bq_d = nc.dram_tensor("bq", [E], F32, kind="ExternalInput")
        bk_d = nc.dram_tensor("bk", [E], F32, kind="ExternalInput")
    if use_bv:
        bv_d = nc.dram_tensor("bv", [E], F32, kind="ExternalInput")
    if use_bo:
        bo_d = nc.dram_tensor("bo", [E], F32, kind="ExternalInput")
    if use_b1:
        b1_d = nc.dram_tensor("b1", [FF], F32, kind="ExternalInput")
    if use_b2:
        b2_d = nc.dram_tensor("b2", [E], F32, kind="ExternalInput")
    if use_g1:
        g1_d = nc.dram_tensor("g1", [E], F32, kind="ExternalInput")
        be1_d = nc.dram_tensor("be1", [E], F32, kind="ExternalInput")
    if use_g2:
        g2_d = nc.dram_tensor("g2", [E], F32, kind="ExternalInput")
        be2_d = nc.dram_tensor("be2", [E], F32, kind="ExternalInput")

    import concourse.bass as bass

    def bcast_ap(dram, n=256):
        return bass.AP(tensor=dram.ap().tensor, offset=0, ap=[[0, 128], [1, n]])

    X = x_d.ap().rearrange("(c t) e -> c (t e)", t=64)      # [256, 16384] bf16
    OUTV = out_d.ap().rearrange("(c t) e -> c t e", t=64)   # [256, 64, 256] f32

    with tile.TileContext(nc) as tc:
        with ExitStack() as ctx:
            const = ctx.enter_context(tc.tile_pool(name="const", bufs=1))

            ident = const.tile([128, 128], BF16)
            nc.sync.dma_start(out=ident, in_=id_d.ap()[:, :])
            wq_t = const.tile([128, 2, 256], BF16)
            wk_t = const.tile([128, 2, 256], BF16)
            wv_t = const.tile([128, 2, 256], BF16)
            wo_t = const.tile([128, 2, 256], BF16)
            for t, d in ((wq_t, wq_d), (wk_t, wk_d), (wv_t, wv_d), (wo_t, wo_d)):
                nc.sync.dma_start(out=t, in_=d.ap().rearrange("(eh k) f -> k eh f", k=128))
            w1_t = const.tile([128, 2, 1024], BF16)
            nc.sync.dma_start(out=w1_t, in_=w1_d.ap().rearrange("(eh k) f -> k eh f", k=128))
            w2_t = const.tile([128, 8, 256], BF16)
            nc.sync.dma_start(out=w2_t, in_=w2_d.ap().rearrange("(fm k) e -> k fm e", k=128))

            if use_bqk:
                bq_t = const.tile([128, 2], F32)
                nc.sync.dma_start(out=bq_t, in_=bq_d.ap().rearrange("(fh p) -> p fh", p=128))
                bk_t = const.tile([128, 2], F32)
                nc.sync.dma_start(out=bk_t, in_=bk_d.ap().rearrange("(fh p) -> p fh", p=128))
            if use_bv:
                bv_bc = const.tile([128, 256], F32)
                nc.sync.dma_start(out=bv_bc, in_=bcast_ap(bv_d))
            if use_bo:
                bo_st = const.tile([128, 2048], F32)
                nc.sync.dma_start(
                    out=bo_st.rearrange("p (g1 wn g2) -> p g1 wn g2", wn=8, g2=16),
                    in_=bass.AP(tensor=bo_d.ap().tensor, offset=0,
                                ap=[[0, 128], [16, 16], [0, 8], [1, 16]]))
            if use_b1:
                b1_t = const.tile([128, 8], F32)
                nc.sync.dma_start(out=b1_t, in_=b1_d.ap().rearrange("(fm p) -> p fm", p=128))
            if use_b2:
                b2_bc = const.tile([128, 256], F32)
                nc.sync.dma_start(out=b2_bc, in_=bcast_ap(b2_d))
            if use_g1:
                g1_bc = const.tile([128, 256], F32)
                nc.sync.dma_start(out=g1_bc, in_=bcast_ap(g1_d))
                be1_bc = const.tile([128, 256], F32)
                nc.sync.dma_start(out=be1_bc, in_=bcast_ap(be1_d))
            if use_g2:
                g2_bc = const.tile([128, 256], F32)
                nc.sync.dma_start(out=g2_bc, in_=bcast_ap(g2_d))
                be2_bc = const.tile([128, 256], F32)
                nc.sync.dma_start(out=be2_bc, in_=bcast_ap(be2_d))

            # attention output, SBUF-resident across phases:
            # [c-part, ct(channel half), hn(stripe), 2048 pixels(g1, w)]
            ys_all = const.tile([128, 2, 8, 2048], BF16)

            def newton_rsqrt(eng_seed, eng_iter, var_ap, n, tagp, pool, iters=1):
                """rstd = 1/sqrt(var + eps) for a [128, n] strided var AP.
                Bit-trick seed on eng_seed (must be DVE); NR iteration
                (mult/add only) can run on Pool."""
                w = pool.tile([128, n], F32, tag=f"nw_w{tagp}", name=f"nw_w{tagp}")
                eng_seed.tensor_scalar(out=w, in0=var_ap, scalar1=1e-5,
                                       scalar2=None, op0=OP.add)
                r = pool.tile([128, n], F32, tag=f"nw_r{tagp}", name=f"nw_r{tagp}")
                eng_seed.tensor_scalar(out=r.bitcast(I32), in0=w.bitcast(I32),
                                       scalar1=1, scalar2=None,
                                       op0=OP.logical_shift_right)
                eng_seed.tensor_scalar(out=r.bitcast(I32), in0=r.bitcast(I32),
                                       scalar1=0xFFFFFFFF, scalar2=None,
                                       op0=OP.bitwise_xor)
                eng_seed.tensor_scalar(out=r.bitcast(I32), in0=r.bitcast(I32),
                                       scalar1=0x5F375A86 + 1, scalar2=None,
                                       op0=OP.add)
                rsq = pool.tile([128, n], F32, tag=f"nw_q{tagp}", name=f"nw_q{tagp}")
                u = pool.tile([128, n], F32, tag=f"nw_u{tagp}", name=f"nw_u{tagp}")
                v = pool.tile([128, n], F32, tag=f"nw_v{tagp}", name=f"nw_v{tagp}")
                for _ in range(iters):
                    eng_iter.tensor_mul(rsq, r, r)
                    eng_iter.tensor_mul(u, rsq, w)
                    eng_iter.tensor_scalar(out=v, in0=u, scalar1=-0.5, scalar2=1.5,
                                           op0=OP.mult, op1=OP.add)
                    eng_iter.tensor_mul(r, r, v)
                return r

            # ================= Phase A: window attention =================
            with ExitStack() as pa:
                xsp = pa.enter_context(tc.tile_pool(name="xsp", bufs=2))
                sa = pa.enter_context(tc.tile_pool(name="sa", bufs=2))
                stp = pa.enter_context(tc.tile_pool(name="stp", bufs=3))
                psR = pa.enter_context(tc.tile_pool(name="psR", bufs=3, space="PSUM"))
                psT = pa.enter_context(tc.tile_pool(name="psT", bufs=2, space="PSUM"))

                xs_tiles = {}

                def load_stripe(hn):
                    t = xsp.tile([128, 2, 2048], BF16, tag="xs", name=f"xs{hn}")
                    for ct in range(2):
                        nc.sync.dma_start(
                            out=t[:, ct, :],
                            in_=X[ct * 128:(ct + 1) * 128, hn * 2048:(hn + 1) * 2048])
                    xs_tiles[hn] = t

                # per-wp state kept across pipeline slots
                st = [dict() for _ in range(NWP)]

                load_stripe(0)
                load_stripe(1)

                def stage_gather(i):
                    """Pool-gather window-pair i's tokens into contiguous t_sb."""
                    hn, wp = divmod(i, 4)
                    if wp == 0 and hn + 2 < Hn:
                        load_stripe(hn + 2)
                    xs = xs_tiles[hn]
                    xv = xs.rearrange("p ct (g1 w) -> p ct g1 w", w=128)
                    t_sb = sa.tile([128, 2, 2, 16, 16], BF16, tag="tsb", name=f"tsb{i}")
                    for ct in range(2):
                        nc.gpsimd.tensor_copy(
                            t_sb[:, ct, :, :, :],
                            xv[:, ct, :, wp * 32:(wp + 1) * 32].rearrange(
                                "p g1 (w g2) -> p w g1 g2", g2=16))
                    st[i]["t_sb"] = t_sb

                def stage_tt(i):
                    """Transpose the 2 windows' tokens: tt = t^T [pix, (w c)]."""
                    t_sb = st[i]["t_sb"].rearrange("p ct w g1 g2 -> p ct (w g1 g2)")
                    tt_ps = psT.tile([128, 2, 2, 256], BF16, tag="t16", name=f"ttp{i}")
                    for eh in range(2):
                        for w in range(2):
                            for ct in range(2):
                                nc.tensor.transpose(
                                    tt_ps[:, eh, w, ct * 128:(ct + 1) * 128],
                                    t_sb[:, ct,
                                         w * 256 + eh * 128:w * 256 + (eh + 1) * 128],
                                    ident)
                    tt = sa.tile([128, 2, 2, 256], BF16, tag="tt", name=f"tt{i}")
                    nc.vector.tensor_copy(tt, tt_ps)
                    st[i]["tt"] = tt

                def stage_q(i):
                    tt = st[i]["tt"]
                    qt_ps = psR.tile([128, 2, 512], F32, tag="r", name=f"qtp{i}")
                    for fh in range(2):
                        for eh in range(2):
                            nc.tensor.matmul(qt_ps[:, fh, :],
                                             lhsT=wq_t[:, eh, fh * 128:(fh + 1) * 128],
                                             rhs=tt[:, eh, :, :],
                                             start=eh == 0, stop=eh == 1)
                    qt = sa.tile([128, 2, 2, 256], BF16, tag="qt", name=f"qt{i}")
                    qv = qt.rearrange("p fh w c -> p fh (w c)")
                    if use_bqk:
                        for fh in range(2):
                            nc.scalar.activation(out=qv[:, fh, :], in_=qt_ps[:, fh, :],
                                                 func=AF.Identity,
                                                 bias=bq_t[:, fh:fh + 1])
                    else:
                        nc.scalar.activation(out=qv, in_=qt_ps, func=AF.Copy)
                    st[i]["qt"] = qt

                def stage_k(i):
                    tt = st[i]["tt"]
                    kt_ps = psR.tile([128, 2, 512], F32, tag="r", name=f"ktp{i}")
                    for fh in range(2):
                        for eh in range(2):
                            nc.tensor.matmul(kt_ps[:, fh, :],
                                             lhsT=wk_t[:, eh, fh * 128:(fh + 1) * 128],
                                             rhs=tt[:, eh, :, :],
                                             start=eh == 0, stop=eh == 1)
                    kt = sa.tile([128, 2, 2, 256], BF16, tag="kt", name=f"kt{i}")
                    kv = kt.rearrange("p fh w c -> p fh (w c)")
                    if use_bqk:
                        for fh in range(2):
                            nc.scalar.activation(out=kv[:, fh, :], in_=kt_ps[:, fh, :],
                                                 func=AF.Identity,
                                                 bias=bk_t[:, fh:fh + 1])
                    else:
                        nc.scalar.activation(out=kv, in_=kt_ps, func=AF.Copy)
                    st[i]["kt"] = kt

                def stage_v(i):
                    tt = st[i]["tt"]
                    v_ps = psR.tile([128, 2, 2, 256], F32, tag="r", name=f"vp{i}")
                    for w in range(2):
                        for ch in range(2):
                            for eh in range(2):
                                nc.tensor.matmul(
                                    v_ps[:, w, ch, :],
                                    lhsT=tt[:, eh, w, ch * 128:(ch + 1) * 128],
                                    rhs=wv_t[:, eh, :],
                                    start=eh == 0, stop=eh == 1)
                    vv = sa.tile([128, 2, 2, 256], BF16, tag="vv", name=f"vv{i}")
                    if use_bv:
                        for w in range(2):
                            for ch in range(2):
                                nc.vector.tensor_add(vv[:, w, ch, :],
                                                     v_ps[:, w, ch, :], bv_bc)
                    else:
                        nc.scalar.activation(out=vv, in_=v_ps, func=AF.Copy)
                    st[i]["vv"] = vv

                def stage_s(i):
                    qt, kt = st[i]["qt"], st[i]["kt"]
                    s_ps = psR.tile([128, 2, 2, 256], F32, tag="r", name=f"sp{i}")
                    for th in range(2):
                        for w in range(2):
                            for fh in range(2):
                                nc.tensor.matmul(
                                    s_ps[:, th, w, :],
                                    lhsT=qt[:, fh, w, th * 128:(th + 1) * 128],
                                    rhs=kt[:, fh, w, :],
                                    start=fh == 0, stop=fh == 1)
                    st[i]["s_ps"] = s_ps

                def stage_exp(i):
                    s_ps = st[i]["s_ps"]
                    aa = sa.tile([128, 2, 2, 256], BF16, tag="aa", name=f"aa{i}")
                    den = stp.tile([128, 2, 2], F32, tag="den", name=f"den{i}")
                    for th in range(2):
                        for w in range(2):
                            nc.scalar.activation(out=aa[:, th, w, :],
                                                 in_=s_ps[:, th, w, :],
                                                 func=AF.Exp,
                                                 accum_out=den[:, th, w:w + 1])
                    st[i]["aa"] = aa
                    st[i]["den"] = den

                def stage_at(i):
                    aa = st[i]["aa"]
                    at_ps = psT.tile([128, 2, 2, 256], BF16, tag="t16", name=f"atp{i}")
                    for t2h in range(2):
                        for w in range(2):
                            for th in range(2):
                                nc.tensor.transpose(
                                    at_ps[:, t2h, w, th * 128:(th + 1) * 128],
                                    aa[:, th, w, t2h * 128:(t2h + 1) * 128],
                                    ident)
                    at = sa.tile([128, 2, 2, 256], BF16, tag="at", name=f"at{i}")
                    nc.vector.tensor_copy(at, at_ps)
                    st[i]["at"] = at

                def stage_ot(i):
                    vv, at = st[i]["vv"], st[i]["at"]
                    ot_ps = psR.tile([128, 2, 2, 256], F32, tag="r", name=f"otp{i}")
                    for w in range(2):
                        for fh in range(2):
                            for t2h in range(2):
                                nc.tensor.matmul(
                                    ot_ps[:, w, fh, :],
                                    lhsT=vv[:, w, t2h, fh * 128:(fh + 1) * 128],
                                    rhs=at[:, t2h, w, :],
                                    start=t2h == 0, stop=t2h == 1)
                    ot = sa.tile([128, 2, 2, 256], BF16, tag="ot", name=f"ot{i}")
                    nc.vector.tensor_copy(ot, ot_ps)
                    st[i]["ot"] = ot

                def stage_o2(i):
                    hn, wp = divmod(i, 4)
                    ot, den = st[i]["ot"], st[i]["den"]
                    o2_ps = psR.tile([128, 2, 2, 256], F32, tag="r", name=f"o2p{i}")
                    for w in range(2):
                        for th in range(2):
                            for fh in range(2):
                                nc.tensor.matmul(
                                    o2_ps[:, w, th, :],
                                    lhsT=ot[:, w, fh, th * 128:(th + 1) * 128],
                                    rhs=wo_t[:, fh, :],
                                    start=fh == 0, stop=fh == 1)
                    rec = stp.tile([128, 2, 2], F32, tag="rec", name=f"rec{i}")
                    nc.vector.reciprocal(rec, den)
                    ysv = ys_all.rearrange("p ct hn (g1 w) -> p ct hn g1 w", w=128)
                    for w in range(2):
                        wn = wp * 2 + w
                        for th in range(2):
                            nc.vector.tensor_scalar(
                                out=ysv[:, th, hn, :, wn * 16:(wn + 1) * 16],
                                in0=o2_ps[:, w, th, :].rearrange(
                                    "p (a b) -> p a b", b=16),
                                scalar1=rec[:, th, w:w + 1], scalar2=None,
                                op0=OP.mult)
                    st[i] = {}

                # software-pipelined slot schedule (2-deep)
                stage_gather(0)
                for slot in range(NWP + 2):
                    if slot < NWP:
                        stage_tt(slot)
                    if slot + 1 < NWP:
                        stage_gather(slot + 1)
                    if 1 <= slot <= NWP:
                        stage_s(slot - 1)
                        stage_exp(slot - 1)
                    if 2 <= slot:
                        stage_o2(slot - 2)
                    if slot < NWP:
                        stage_q(slot)
                        stage_k(slot)
                    if 1 <= slot <= NWP:
                        stage_at(slot - 1)
                    if slot < NWP:
                        stage_v(slot)
                    if 1 <= slot <= NWP:
                        stage_ot(slot - 1)

                if use_bo:
                    for ct in range(2):
                        for hn in range(Hn):
                            nc.gpsimd.tensor_add(
                                ys_all[:, ct, hn, :],
                                ys_all[:, ct, hn, :].bitcast(BF16), bo_st)

            # ================= Phase B: FFN + 2 post-LNs =================
            with ExitStack() as pb:
                sb = pb.enter_context(tc.tile_pool(name="sb", bufs=2))
                msc = pb.enter_context(tc.tile_pool(name="msc", bufs=2))
                psH = pb.enter_context(tc.tile_pool(name="psH", bufs=2, space="PSUM"))
                psZ = pb.enter_context(tc.tile_pool(name="psZ", bufs=2, space="PSUM"))
                psY = pb.enter_context(tc.tile_pool(name="psY", bufs=1, space="PSUM"))

                fst = [dict() for _ in range(NNB)]

                def chunks_of(nb):
                    # 4 token chunks (ct, j) within stripe hn
                    return nb // 4, [(q // 8, q % 8) for q in
                                     range((nb % 4) * 4, (nb % 4) * 4 + 4)]

                def stage_yt(nb):
                    """Transpose 4 token chunks into feature-major yt."""
                    hn, chunks = chunks_of(nb)
                    yt_ps = psY.tile([128, 2, 512], BF16, tag="yt16", name=f"ytp{nb}")
                    for eh in range(2):
                        for pos, (ct, j) in enumerate(chunks):
                            nc.tensor.transpose(
                                yt_ps[:, eh, pos * 128:(pos + 1) * 128],
                                ys_all[:, ct, hn,
                                       j * 256 + eh * 128:j * 256 + (eh + 1) * 128],
                                ident)
                    yt = sb.tile([128, 2, 512], BF16, tag="yt", name=f"yt{nb}")
                    nc.vector.tensor_copy(yt, yt_ps)
                    fst[nb]["yt"] = yt

                def stage_h(nb):
                    yt = fst[nb]["yt"]
                    hh = sb.tile([128, 8, 512], BF16, tag="hh", name=f"hh{nb}")
                    for fm in range(8):
                        h_ps = psH.tile([128, 512], F32, tag="h", name=f"hp{nb}_{fm}")
                        for eh in range(2):
                            nc.tensor.matmul(
                                h_ps,
                                lhsT=w1_t[:, eh, fm * 128:(fm + 1) * 128],
                                rhs=yt[:, eh, :],
                                start=eh == 0, stop=eh == 1)
                        nc.scalar.activation(out=hh[:, fm, :], in_=h_ps,
                                             func=AF.Gelu,
                                             bias=b1_t[:, fm:fm + 1] if use_b1
                                             else 0.0)
                    fst[nb]["hh"] = hh

                def stage_f(nb):
                    hh = fst[nb]["hh"]
                    z_ps = psZ.tile([128, 4, 256], F32, tag="z", name=f"zp{nb}")
                    for pos in range(4):
                        for fm in range(8):
                            nc.tensor.matmul(
                                z_ps[:, pos, :],
                                lhsT=hh[:, fm, pos * 128:(pos + 1) * 128],
                                rhs=w2_t[:, fm, :],
                                start=fm == 0, stop=fm == 7)
                    fst[nb]["z_ps"] = z_ps

                def stage_ln(nb):
                    hn, chunks = chunks_of(nb)
                    z_ps = fst[nb]["z_ps"]
                    if use_b2:
                        zb = sb.tile([128, 4, 256], F32, tag="zb", name=f"zb{nb}")
                        for pos in range(4):
                            nc.vector.tensor_add(zb[:, pos, :], z_ps[:, pos, :], b2_bc)
                        z_src = zb
                    else:
                        z_src = z_ps

                    mvs1 = msc.tile([128, 4, 2], F32, tag="mvs1", name=f"mv1{nb}")
                    bst1 = msc.tile([128, 4, 6], F32, tag="bst1", name=f"b1s{nb}")
                    for pos in range(4):
                        nc.vector.bn_stats(out=bst1[:, pos, :], in_=z_src[:, pos, :])
                        nc.vector.bn_aggr(out=mvs1[:, pos, :], in_=bst1[:, pos, :])
                    rs1 = newton_rsqrt(nc.vector, nc.gpsimd, mvs1[:, :, 1], 4,
                                       "1", msc)
                    nb1 = msc.tile([128, 4], F32, tag="nb1", name=f"nb1{nb}")
                    # nb1 = -mean1 * rs1
                    nc.vector.scalar_tensor_tensor(
                        out=nb1, in0=mvs1[:, :, 0], scalar=-1.0, in1=rs1,
                        op0=OP.mult, op1=OP.mult)

                    y2 = sb.tile([128, 4, 256], F32, tag="y2", name=f"y2{nb}")
                    if use_g1:
                        # ln1 = z*rs1 + nb1 on ACT; *g1 + be1; y2 = ln1 + ys
                        ln1 = sb.tile([128, 4, 256], BF16, tag="ln1", name=f"ln1{nb}")
                        for pos, (ct, j) in enumerate(chunks):
                            nc.scalar.activation(out=ln1[:, pos, :],
                                                 in_=z_src[:, pos, :],
                                                 func=AF.Identity,
                                                 scale=rs1[:, pos:pos + 1],
                                                 bias=nb1[:, pos:pos + 1])
                            nc.vector.tensor_mul(ln1[:, pos, :], ln1[:, pos, :], g1_bc)
                            nc.vector.tensor_add(ln1[:, pos, :], ln1[:, pos, :], be1_bc)
                            nc.gpsimd.tensor_add(
                                y2[:, pos, :], ln1[:, pos, :],
                                ys_all[:, ct, hn, j * 256:(j + 1) * 256])
                    else:
                        # chunks 0,1: fused y2 = (z*rs1 + nb1) + ys on DVE;
                        # chunks 2,3: ln1 on ACT + add on Pool (engine balance)
                        from concourse.dve_ops import AFFINE_THEN_ADD
                        ln1 = sb.tile([128, 2, 256], BF16, tag="ln1", name=f"ln1{nb}")
                        for pos, (ct, j) in enumerate(chunks):
                            if pos < 2:
                                nc.vector._custom_dve(
                                    AFFINE_THEN_ADD,
                                    out=y2[:, pos, :], in0=z_src[:, pos, :],
                                    in1=ys_all[:, ct, hn, j * 256:(j + 1) * 256],
                                    s0=rs1[:, pos:pos + 1], s1=nb1[:, pos:pos + 1])
                            else:
                                nc.scalar.activation(out=ln1[:, pos - 2, :],
                                                     in_=z_src[:, pos, :],
                                                     func=AF.Identity,
                                                     scale=rs1[:, pos:pos + 1],
                                                     bias=nb1[:, pos:pos + 1])
                                nc.gpsimd.tensor_add(
                                    y2[:, pos, :], ln1[:, pos - 2, :],
                                    ys_all[:, ct, hn, j * 256:(j + 1) * 256])

                    mvs2 = msc.tile([128, 4, 2], F32, tag="mvs2", name=f"mv2{nb}")
                    bst2 = msc.tile([128, 4, 6], F32, tag="bst2", name=f"b2s{nb}")
                    for pos in range(4):
                        nc.vector.bn_stats(out=bst2[:, pos, :], in_=y2[:, pos, :])
                        nc.vector.bn_aggr(out=mvs2[:, pos, :], in_=bst2[:, pos, :])
                    rs2 = newton_rsqrt(nc.vector, nc.gpsimd, mvs2[:, :, 1], 4,
                                       "2", msc)

                    outt = sb.tile([128, 4, 256], F32, tag="outt", name=f"out{nb}")
                    if use_g2:
                        nb2 = msc.tile([128, 4], F32, tag="nb2", name=f"nb2{nb}")
                        nc.vector.scalar_tensor_tensor(
                            out=nb2, in0=mvs2[:, :, 0], scalar=-1.0, in1=rs2,
                            op0=OP.mult, op1=OP.mult)
                        ln2 = sb.tile([128, 4, 256], F32, tag="ln2", name=f"ln2{nb}")
                        for pos in range(4):
                            nc.vector.tensor_scalar(
                                out=ln2[:, pos, :], in0=y2[:, pos, :],
                                scalar1=rs2[:, pos:pos + 1],
                                scalar2=nb2[:, pos:pos + 1],
                                op0=OP.mult, op1=OP.add)
                            nc.vector.tensor_mul(ln2[:, pos, :], ln2[:, pos, :], g2_bc)
                            nc.vector.tensor_add(ln2[:, pos, :], ln2[:, pos, :], be2_bc)
                            nc.gpsimd.tensor_add(outt[:, pos, :], ln2[:, pos, :],
                                                 y2[:, pos, :])
                    else:
                        # outt = y2*(1+rs2) - mean2*rs2, fused
                        sc1 = msc.tile([128, 4], F32, tag="sc1", name=f"sc1{nb}")
                        nc.vector.tensor_scalar(out=sc1, in0=rs2, scalar1=1.0,
                                                scalar2=None, op0=OP.add)
                        sc2 = msc.tile([128, 4], F32, tag="sc2", name=f"sc2{nb}")
                        nc.vector.scalar_tensor_tensor(
                            out=sc2, in0=mvs2[:, :, 0], scalar=-1.0, in1=rs2,
                            op0=OP.mult, op1=OP.mult)
                        for pos in range(4):
                            nc.gpsimd.tensor_scalar(
                                out=outt[:, pos, :], in0=y2[:, pos, :],
                                scalar1=sc1[:, pos:pos + 1],
                                scalar2=sc2[:, pos:pos + 1],
                                op0=OP.mult, op1=OP.add)
                    for pos, (ct, j) in enumerate(chunks):
                        nc.sync.dma_start(
                            out=OUTV[ct * 128:(ct + 1) * 128, hn * 8 + j, :],
                            in_=outt[:, pos, :])
                    fst[nb] = {}

                for slot in range(NNB + 2):
                    if slot < NNB:
                        stage_yt(slot)
                    if 1 <= slot <= NNB:
                        stage_h(slot - 1)
                    if 2 <= slot:
                        stage_f(slot - 2)
                        stage_ln(slot - 2)

    nc.compile()
    return nc


def _get_program(flags):
    if flags not in _CACHE:
        _CACHE[flags] = _build(flags)
    return _CACHE[flags]


def kernel(**inputs):
    import ml_dtypes
    BF = ml_dtypes.bfloat16

    x = np.asarray(inputs["x"], np.float32)
    Wq = np.asarray(inputs["Wq"], np.float32)
    Wk = np.asarray(inputs["Wk"], np.float32)
    Wv = np.asarray(inputs["Wv"], np.float32)
    Wo = np.asarray(inputs["Wo"], np.float32)
    W1 = np.asarray(inputs["W1"], np.float32)
    W2 = np.asarray(inputs["W2"], np.float32)
    bq = np.asarray(inputs["bq"], np.float32)
    bk = np.asarray(inputs["bk"], np.float32)
    bv = np.asarray(inputs["bv"], np.float32)
    bo = np.asarray(inputs["bo"], np.float32)
    b1 = np.asarray(inputs["b1"], np.float32)
    b2 = np.asarray(inputs["b2"], np.float32)
    g1 = np.asarray(inputs["g1"], np.float32)
    be1 = np.asarray(inputs["be1"], np.float32)
    g2 = np.asarray(inputs["g2"], np.float32)
    be2 = np.asarray(inputs["be2"], np.float32)

    flags = (
        bool(bq.any() or bk.any()),
        bool(bv.any()),
        bool(bo.any()),
        bool(b1.any()),
        bool(b2.any()),
        bool((g1 != 1.0).any() or be1.any()),
        bool((g2 != 1.0).any() or be2.any()),
    )
    nc = _get_program(flags)

    scale = 1.0 / np.sqrt(np.float32(E))
    base = {
        "wq": (Wq * scale).astype(BF),
        "wk": Wk.astype(BF),
        "wv": Wv.astype(BF),
        "wo": Wo.astype(BF),
        "w1": W1.astype(BF),
        "w2": W2.astype(BF),
        "ident": np.eye(128, dtype=np.float32).astype(BF),
    }
    use_bqk, use_bv, use_bo, use_b1, use_b2, use_g1, use_g2 = flags
    if use_bqk:
        base["bq"] = bq * scale
        base["bk"] = bk
    if use_bv:
        base["bv"] = bv
    if use_bo:
        base["bo"] = bo
    if use_b1:
        base["b1"] = b1
    if use_b2:
        base["b2"] = b2
    if use_g1:
        base["g1"] = g1
        base["be1"] = be1
    if use_g2:
        base["g2"] = g2
        base["be2"] = be2

    in_maps = [dict(base, x=x[b].astype(BF)) for b in range(B)]

    from concourse.bass_utils import run_bass_kernel_spmd

    trace = os.environ.get("TRN_TRACE") == "1"
    tmpdir = os.environ.get("TRN_TRACE_DIR") or None
    res = run_bass_kernel_spmd(nc, in_maps, list(range(B)), trace=trace,
                               tmpdir=tmpdir)
    kernel.last_exec_time_ns = res.exec_time_ns
    kernel.last_profile_json = res.profile_json
    kernel.last_trace = res.instructions_and_trace
    return np.stack([r["out"] for r in res.results], axis=0)


# revision 40
# speedup vs baseline: 1.1000x; 1.1000x over previous
"""GridTransformerBlock TRN2 kernel (v2).

Sharding: batch-parallel over B=8 -> one batch per NeuronCore, zero
collectives.

v2 design (vs v1 baseline at 1.147ms):
- bf16 operands everywhere (weights + activations). At N>=256 the PE streams
  1 row/cycle for both f32r and bf16, but bf16 gets Fast Weight Load
  (f32r LDWEIGHTS was 226ns x 3584 = 810us of PE weight-path time), 1.0
  (vs 1.5) cycles/row transposes, and halves SBUF/DMA traffic.
- Phase split: ALL window attention first (ACT table = exp only), then ALL
  FFN (ACT table = gelu only). v1 alternated per stripe and paid ~44
  ACT_TABLE_LOADs. ys (8.4MB bf16) stays SBUF-resident between phases.
- Window-pair batching: Q/K projections stream N=512 through one weight
  load; software-pipelined slot schedule keeps the PE fed across the
  transpose->copy->matmul dependency chains (v1 had 590us of HAM throttle
  from PE idle gaps).
- FFN W2 matmul emits token-major output directly (lhsT = gelu chunks),
  killing the f^T transposes + copies; LN epilogue fused into
  tensor_scalar ops; 2-iteration Newton rsqrt.
"""

import os
import sys
import numpy as np

for _p in ("/opt/trn_rl_repo", "/root/.axon_site/_ro/trn_rl_repo"):
    if _p not in sys.path and os.path.isdir(_p):
        sys.path.insert(0, _p)

B, S, E, FF = 8, 16384, 256, 1024
H, W, G = 128, 128, 16
Hn, Wn = 8, 8
NWP = Hn * (Wn // 2)   # 32 window-pairs
NNB = 32               # 32 FFN blocks of 512 tokens

_CACHE = {}


def _build(flags):
    use_bqk, use_bv, use_bo, use_b1, use_b2, use_g1, use_g2 = flags
    import concourse.bacc as bacc
    import concourse.mybir as mybir
    import concourse.tile as tile
    from contextlib import ExitStack

    F32 = mybir.dt.float32
    BF16 = mybir.dt.bfloat16
    I32 = mybir.dt.int32
    AF = mybir.ActivationFunctionType
    OP = mybir.AluOpType

    nc = bacc.Bacc("TRN2", target_bir_lowering=False, debug=False, num_devices=8)

    x_d = nc.dram_tensor("x", [S, E], BF16, kind="ExternalInput")
    wq_d = nc.dram_tensor("wq", [E, E], BF16, kind="ExternalInput")
    wk_d = nc.dram_tensor("wk", [E, E], BF16, kind="ExternalInput")
    wv_d = nc.dram_tensor("wv", [E, E], BF16, kind="ExternalInput")
    wo_d = nc.dram_tensor("wo", [E, E], BF16, kind="ExternalInput")
    w1_d = nc.dram_tensor("w1", [E, FF], BF16, kind="ExternalInput")
    w2_d = nc.dram_tensor("w2", [FF, E], BF16, kind="ExternalInput")
    id_d = nc.dram_tensor("ident", [128, 128], BF16, kind="ExternalInput")
    out_d = nc.dram_tensor("out", [S, E], F32, kind="ExternalOutput")
    if use_bqk:
        bq_d = nc.dram_tensor("bq", [E], F32, kind="ExternalInput")
        bk_d = nc.dram_tensor("bk", [E], F32, kind="ExternalInput")
    if use_bv:
        bv_d = nc.dram_tensor("bv", [E], F32, kind="ExternalInput")
    if use_bo:
        bo_d = nc.dram_tensor("bo", [E], F32, kind="ExternalInput")
    if use_b1:
        b1_d = nc.dram_tensor("b1", [FF], F32, kind="ExternalInput")
    if use_b2:
        b2_d = nc.dram_tensor("b2", [E], F32, kind="ExternalInput")
    if use_g1:
        g1_d = nc.dram_tensor("g1", [E], F32, kind="ExternalInput")
        be1_d = nc.dram_tensor("be1", [E], F32, kind="ExternalInput")
    if use_g2:
        g2_d = nc.dram_tensor("g2", [E], F32, kind="ExternalInput")
        be2_d = nc.dram_tensor("be2", [E], F32, kind="ExternalInput")

    import concourse.bass as bass

    def bcast_ap(dram, n=256):
        return bass.AP(tensor=dram.ap().tensor, offset=0, ap=[[0, 128], [1, n]])

    X = x_d.ap().rearrange("(c t) e -> c (t e)", t=64)      # [256, 16384] bf16
    OUTV = out_d.ap().rearrange("(c t) e -> c t e", t=64)   # [256, 64, 256] f32

    with tile.TileContext(nc) as tc:
        with ExitStack() as ctx:
            const = ctx.enter_context(tc.tile_pool(name="const", bufs=1))

            ident = const.tile([128, 128], BF16)
            nc.sync.dma_start(out=ident, in_=id_d.ap()[:, :])
            wq_t = const.tile([128, 2, 256], BF16)
            wk_t = const.tile([128, 2, 256], BF16)
            wv_t = const.tile([128, 2, 256], BF16)
            wo_t = const.tile([128, 2, 256], BF16)
            for t, d in ((wq_t, wq_d), (wk_t, wk_d), (wv_t, wv_d), (wo_t, wo_d)):
                nc.sync.dma_start(out=t, in_=d.ap().rearrange("(eh k) f -> k eh f", k=128))
            w1_t = const.tile([128, 2, 1024], BF16)
            nc.sync.dma_start(out=w1_t, in_=w1_d.ap().rearrange("(eh k) f -> k eh f", k=128))
            w2_t = const.tile([128, 8, 256], BF16)
            nc.sync.dma_start(out=w2_t, in_=w2_d.ap().rearrange("(fm k) e -> k fm e", k=128))

            if use_bqk:
                bq_t = const.tile([128, 2], F32)
                nc.sync.dma_start(out=bq_t, in_=bq_d.ap().rearrange("(fh p) -> p fh", p=128))
                bk_t = const.tile([128, 2], F32)
                nc.sync.dma_start(out=bk_t, in_=bk_d.ap().rearrange("(fh p) -> p fh", p=128))
            if use_bv:
                bv_bc = const.tile([128, 256], F32)
                nc.sync.dma_start(out=bv_bc, in_=bcast_ap(bv_d))
            if use_bo:
                bo_st = const.tile([128, 2048], F32)
                nc.sync.dma_start(
                    out=bo_st.rearrange("p (g1 wn g2) -> p g1 wn g2", wn=8, g2=16),
                    in_=bass.AP(tensor=bo_d.ap().tensor, offset=0,
                                ap=[[0, 128], [16, 16], [0, 8], [1, 16]]))
            if use_b1:
                b1_t = const.tile([128, 8], F32)
                nc.sync.dma_start(out=b1_t, in_=b1_d.ap().rearrange("(fm p) -> p fm", p=128))
            if use_b2:
                b2_bc = const.tile([128, 256], F32)
                nc.sync.dma_start(out=b2_bc, in_=bcast_ap(b2_d))
            if use_g1:
                g1_bc = const.tile([128, 256], F32)
                nc.sync.dma_start(out=g1_bc, in_=bcast_ap(g1_d))
                be1_bc = const.tile([128, 256], F32)
                nc.sync.dma_start(out=be1_bc, in_=bcast_ap(be1_d))
            if use_g2:
                g2_bc = const.tile([128, 256], F32)
                nc.sync.dma_start(out=g2_bc, in_=bcast_ap(g2_d))
                be2_bc = const.tile([128, 256], F32)
                nc.sync.dma_start(out=be2_bc, in_=bcast_ap(be2_d))

            # attention output, SBUF-resident across phases:
            # [c-part, ct(channel half), hn(stripe), 2048 pixels(g1, w)]
            ys_all = const.tile([128, 2, 8, 2048], BF16)

            def newton_rsqrt(eng_seed, eng_iter, var_ap, n, tagp, pool, iters=1):
                """rstd = 1/sqrt(var + eps) for a [128, n] strided var AP.
                Bit-trick seed on eng_seed (must be DVE); NR iteration
                (mult/add only) can run on Pool."""
                w = pool.tile([128, n], F32, tag=f"nw_w{tagp}", name=f"nw_w{tagp}")
                eng_seed.tensor_scalar(out=w, in0=var_ap, scalar1=1e-5,
                                       scalar2=None, op0=OP.add)
                r = pool.tile([128, n], F32, tag=f"nw_r{tagp}", name=f"nw_r{tagp}")
                eng_seed.tensor_scalar(out=r.bitcast(I32), in0=w.bitcast(I32),
                                       scalar1=1, scalar2=None,
                                       op0=OP.logical_shift_right)
                eng_seed.tensor_scalar(out=r.bitcast(I32), in0=r.bitcast(I32),
                                       scalar1=0xFFFFFFFF, scalar2=None,
                                       op0=OP.bitwise_xor)
                eng_seed.tensor_scalar(out=r.bitcast(I32), in0=r.bitcast(I32),
                                       scalar1=0x5F375A86 + 1, scalar2=None,
                                       op0=OP.add)
                rsq = pool.tile([128, n], F32, tag=f"nw_q{tagp}", name=f"nw_q{tagp}")
                u = pool.tile([128, n], F32, tag=f"nw_u{tagp}", name=f"nw_u{tagp}")
                v = pool.tile([128, n], F32, tag=f"nw_v{tagp}", name=f"nw_v{tagp}")
                for _ in range(iters):
                    eng_iter.tensor_mul(rsq, r, r)
                    eng_iter.tensor_mul(u, rsq, w)
                    eng_iter.tensor_scalar(out=v, in0=u, scalar1=-0.5, scalar2=1.5,
                                           op0=OP.mult, op1=OP.add)
                    eng_iter.tensor_mul(r, r, v)
                return r

            # ================= Phase A: window attention =================
            with ExitStack() as pa:
                xsp = pa.enter_context(tc.tile_pool(name="xsp", bufs=2))
                sa = pa.enter_context(tc.tile_pool(name="sa", bufs=2))
                stp = pa.enter_context(tc.tile_pool(name="stp", bufs=3))
                psR = pa.enter_context(tc.tile_pool(name="psR", bufs=3, space="PSUM"))
                psT = pa.enter_context(tc.tile_pool(name="psT", bufs=2, space="PSUM"))

                xs_tiles = {}

                def load_stripe(hn):
                    t = xsp.tile([128, 2, 2048], BF16, tag="xs", name=f"xs{hn}")
                    for ct in range(2):
                        nc.sync.dma_start(
                            out=t[:, ct, :],
                            in_=X[ct * 128:(ct + 1) * 128, hn * 2048:(hn + 1) * 2048])
                    xs_tiles[hn] = t

                # per-wp state kept across pipeline slots
                st = [dict() for _ in range(NWP)]

                load_stripe(0)
                load_stripe(1)

                def stage_gather(i):
                    """Pool-gather window-pair i's tokens into contiguous t_sb."""
                    hn, wp = divmod(i, 4)
                    if wp == 0 and hn + 2 < Hn:
                        load_stripe(hn + 2)
                    xs = xs_tiles[hn]
                    xv = xs.rearrange("p ct (g1 w) -> p ct g1 w", w=128)
                    t_sb = sa.tile([128, 2, 2, 16, 16], BF16, tag="tsb", name=f"tsb{i}")
                    for ct in range(2):
                        nc.gpsimd.tensor_copy(
                            t_sb[:, ct, :, :, :],
                            xv[:, ct, :, wp * 32:(wp + 1) * 32].rearrange(
                                "p g1 (w g2) -> p w g1 g2", g2=16))
                    st[i]["t_sb"] = t_sb

                def stage_tt(i):
                    """Transpose the 2 windows' tokens: tt = t^T [pix, (w c)]."""
                    t_sb = st[i]["t_sb"].rearrange("p ct w g1 g2 -> p ct (w g1 g2)")
                    tt_ps = psT.tile([128, 2, 2, 256], BF16, tag="t16", name=f"ttp{i}")
                    for eh in range(2):
                        for w in range(2):
                            for ct in range(2):
                                nc.tensor.transpose(
                                    tt_ps[:, eh, w, ct * 128:(ct + 1) * 128],
                                    t_sb[:, ct,
                                         w * 256 + eh * 128:w * 256 + (eh + 1) * 128],
                                    ident)
                    tt = sa.tile([128, 2, 2, 256], BF16, tag="tt", name=f"tt{i}")
                    nc.vector.tensor_copy(tt, tt_ps)
                    st[i]["tt"] = tt

                def stage_q(i):
                    tt = st[i]["tt"]
                    qt_ps = psR.tile([128, 2, 512], F32, tag="r", name=f"qtp{i}")
                    for fh in range(2):
                        for eh in range(2):
                            nc.tensor.matmul(qt_ps[:, fh, :],
                                             lhsT=wq_t[:, eh, fh * 128:(fh + 1) * 128],
                                             rhs=tt[:, eh, :, :],
                                             start=eh == 0, stop=eh == 1)
                    qt = sa.tile([128, 2, 2, 256], BF16, tag="qt", name=f"qt{i}")
                    qv = qt.rearrange("p fh w c -> p fh (w c)")
                    if use_bqk:
                        for fh in range(2):
                            nc.scalar.activation(out=qv[:, fh, :], in_=qt_ps[:, fh, :],
                                                 func=AF.Identity,
                                                 bias=bq_t[:, fh:fh + 1])
                    else:
                        nc.scalar.activation(out=qv, in_=qt_ps, func=AF.Copy)
                    st[i]["qt"] = qt

                def stage_k(i):
                    tt = st[i]["tt"]
                    kt_ps = psR.tile([128, 2, 512], F32, tag="r", name=f"ktp{i}")
                    for fh in range(2):
                        for eh in range(2):
                            nc.tensor.matmul(kt_ps[:, fh, :],
                                             lhsT=wk_t[:, eh, fh * 128:(fh + 1) * 128],
                                             rhs=tt[:, eh, :, :],
                                             start=eh == 0, stop=eh == 1)
                    kt = sa.tile([128, 2, 2, 256], BF16, tag="kt", name=f"kt{i}")
                    kv = kt.rearrange("p fh w c -> p fh (w c)")
                    if use_bqk:
                        for fh in range(2):
                            nc.vector.tensor_scalar(
                                out=kv[:, fh, :], in0=kt_ps[:, fh, :],
                                scalar1=bk_t[:, fh:fh + 1], scalar2=None,
                                op0=OP.add)
                    else:
                        nc.vector.tensor_copy(kv, kt_ps)
                    st[i]["kt"] = kt

                def stage_v(i):
                    tt = st[i]["tt"]
                    v_ps = psR.tile([128, 2, 2, 256], F32, tag="r", name=f"vp{i}")
                    for w in range(2):
                        for ch in range(2):
                            for eh in range(2):
                                nc.tensor.matmul(
                                    v_ps[:, w, ch, :],
                                    lhsT=tt[:, eh, w, ch * 128:(ch + 1) * 128],
                                    rhs=wv_t[:, eh, :],
                                    start=eh == 0, stop=eh == 1)
                    vv = sa.tile([128, 2, 2, 256], BF16, tag="vv", name=f"vv{i}",
                                 bufs=3)
                    if use_bv:
                        for w in range(2):
                            for ch in range(2):
                                nc.vector.tensor_add(vv[:, w, ch, :],
                                                     v_ps[:, w, ch, :], bv_bc)
                    else:
                        nc.scalar.activation(out=vv, in_=v_ps, func=AF.Copy)
                    st[i]["vv"] = vv

                def stage_s(i):
                    qt, kt = st[i]["qt"], st[i]["kt"]
                    s_ps = psR.tile([128, 2, 2, 256], F32, tag="r", name=f"sp{i}")
                    for th in range(2):
                        for w in range(2):
                            for fh in range(2):
                                nc.tensor.matmul(
                                    s_ps[:, th, w, :],
                                    lhsT=qt[:, fh, w, th * 128:(th + 1) * 128],
                                    rhs=kt[:, fh, w, :],
                                    start=fh == 0, stop=fh == 1)
                    st[i]["s_ps"] = s_ps

                def stage_exp(i):
                    s_ps = st[i]["s_ps"]
                    aa = sa.tile([128, 2, 2, 256], BF16, tag="aa", name=f"aa{i}")
                    den = stp.tile([128, 2, 2], F32, tag="den", name=f"den{i}")
                    for th in range(2):
                        for w in range(2):
                            nc.scalar.activation(out=aa[:, th, w, :],
                                                 in_=s_ps[:, th, w, :],
                                                 func=AF.Exp,
                                                 accum_out=den[:, th, w:w + 1])
                    st[i]["aa"] = aa
                    st[i]["den"] = den

                def stage_at(i):
                    aa = st[i]["aa"]
                    at_ps = psT.tile([128, 2, 2, 256], BF16, tag="t16", name=f"atp{i}")
                    for t2h in range(2):
                        for w in range(2):
                            for th in range(2):
                                nc.tensor.transpose(
                                    at_ps[:, t2h, w, th * 128:(th + 1) * 128],
                                    aa[:, th, w, t2h * 128:(t2h + 1) * 128],
                                    ident)
                    at = sa.tile([128, 2, 2, 256], BF16, tag="at", name=f"at{i}")
                    nc.vector.tensor_copy(at, at_ps)
                    st[i]["at"] = at

                def stage_ot(i):
                    vv, at = st[i]["vv"], st[i]["at"]
                    ot_ps = psR.tile([128, 2, 2, 256], F32, tag="r", name=f"otp{i}")
                    for w in range(2):
                        for fh in range(2):
                            for t2h in range(2):
                                nc.tensor.matmul(
                                    ot_ps[:, w, fh, :],
                                    lhsT=vv[:, w, t2h, fh * 128:(fh + 1) * 128],
                                    rhs=at[:, t2h, w, :],
                                    start=t2h == 0, stop=t2h == 1)
                    ot = sa.tile([128, 2, 2, 256], BF16, tag="ot", name=f"ot{i}")
                    nc.vector.tensor_copy(ot, ot_ps)
                    st[i]["ot"] = ot

                def stage_o2(i):
                    hn, wp = divmod(i, 4)
                    ot, den = st[i]["ot"], st[i]["den"]
                    o2_ps = psR.tile([128, 2, 2, 256], F32, tag="r", name=f"o2p{i}")
                    for w in range(2):
                        for th in range(2):
                            for fh in range(2):
                                nc.tensor.matmul(
                                    o2_ps[:, w, th, :],
                                    lhsT=ot[:, w, fh, th * 128:(th + 1) * 128],
                                    rhs=wo_t[:, fh, :],
                                    start=fh == 0, stop=fh == 1)
                    rec = stp.tile([128, 2, 2], F32, tag="rec", name=f"rec{i}")
                    nc.vector.reciprocal(rec, den)
                    ysv = ys_all.rearrange("p ct hn (g1 w) -> p ct hn g1 w", w=128)
                    for w in range(2):
                        wn = wp * 2 + w
                        for th in range(2):
                            nc.vector.tensor_scalar(
                                out=ysv[:, th, hn, :, wn * 16:(wn + 1) * 16],
                                in0=o2_ps[:, w, th, :].rearrange(
                                    "p (a b) -> p a b", b=16),
                                scalar1=rec[:, th, w:w + 1], scalar2=None,
                                op0=OP.mult)
                    st[i] = {}

                # software-pipelined slot schedule (3-deep: at/ot one slot
                # after exp, o2 one more, so each PSUM->SBUF copy has a full
                # slot of slack before its consumer)
                stage_gather(0)
                for slot in range(NWP + 3):
                    if slot < NWP:
                        stage_tt(slot)
                    if slot + 1 < NWP:
                        stage_gather(slot + 1)
                    if 2 <= slot <= NWP + 1:
                        stage_at(slot - 2)
                    if 1 <= slot <= NWP:
                        stage_s(slot - 1)
                        stage_exp(slot - 1)
                    if 3 <= slot:
                        stage_o2(slot - 3)
                    if slot < NWP:
                        stage_q(slot)
                        stage_k(slot)
                        stage_v(slot)
                    if 2 <= slot <= NWP + 1:
                        stage_ot(slot - 2)

                if use_bo:
                    for ct in range(2):
                        for hn in range(Hn):
                            nc.gpsimd.tensor_add(
                                ys_all[:, ct, hn, :],
                                ys_all[:, ct, hn, :].bitcast(BF16), bo_st)

            # ================= Phase B: FFN + 2 post-LNs =================
            with ExitStack() as pb:
                sb = pb.enter_context(tc.tile_pool(name="sb", bufs=2))
                msc = pb.enter_context(tc.tile_pool(name="msc", bufs=2))
                psH = pb.enter_context(tc.tile_pool(name="psH", bufs=4, space="PSUM"))
                psZ = pb.enter_context(tc.tile_pool(name="psZ", bufs=1, space="PSUM"))
                psY = pb.enter_context(tc.tile_pool(name="psY", bufs=1, space="PSUM"))

                fst = [dict() for _ in range(NNB)]

                def chunks_of(nb):
                    # 4 token chunks (ct, j) within stripe hn
                    return nb // 4, [(q // 8, q % 8) for q in
                                     range((nb % 4) * 4, (nb % 4) * 4 + 4)]

                def stage_yt(nb):
                    """Transpose 4 token chunks into feature-major yt."""
                    hn, chunks = chunks_of(nb)
                    yt_ps = psY.tile([128, 2, 512], BF16, tag="yt16", name=f"ytp{nb}")
                    for eh in range(2):
                        for pos, (ct, j) in enumerate(chunks):
                            nc.tensor.transpose(
                                yt_ps[:, eh, pos * 128:(pos + 1) * 128],
                                ys_all[:, ct, hn,
                                       j * 256 + eh * 128:j * 256 + (eh + 1) * 128],
                                ident)
                    yt = sb.tile([128, 2, 512], BF16, tag="yt", name=f"yt{nb}")
                    nc.scalar.activation(out=yt, in_=yt_ps, func=AF.Copy)
                    fst[nb]["yt"] = yt

                def stage_h(nb):
                    yt = fst[nb]["yt"]
                    hh = sb.tile([128, 8, 512], BF16, tag="hh", name=f"hh{nb}")
                    for fm in range(8):
                        h_ps = psH.tile([128, 512], F32, tag="h", name=f"hp{nb}_{fm}")
                        for eh in range(2):
                            nc.tensor.matmul(
                                h_ps,
                                lhsT=w1_t[:, eh, fm * 128:(fm + 1) * 128],
                                rhs=yt[:, eh, :],
                                start=eh == 0, stop=eh == 1)
                        nc.scalar.activation(out=hh[:, fm, :], in_=h_ps,
                                             func=AF.Gelu,
                                             bias=b1_t[:, fm:fm + 1] if use_b1
                                             else 0.0)
                    fst[nb]["hh"] = hh

                def stage_f(nb):
                    hh = fst[nb]["hh"]
                    z_ps = psZ.tile([128, 4, 256], F32, tag="z", name=f"zp{nb}")
                    for pos in range(4):
                        for fm in range(8):
                            nc.tensor.matmul(
                                z_ps[:, pos, :],
                                lhsT=hh[:, fm, pos * 128:(pos + 1) * 128],
                                rhs=w2_t[:, fm, :],
                                start=fm == 0, stop=fm == 7)
                    fst[nb]["z_ps"] = z_ps

                def stage_ln(nb):
                    hn, chunks = chunks_of(nb)
                    z_ps = fst[nb]["z_ps"]
                    if use_b2:
                        zb = sb.tile([128, 4, 256], F32, tag="zb", name=f"zb{nb}")
                        for pos in range(4):
                            nc.vector.tensor_add(zb[:, pos, :], z_ps[:, pos, :], b2_bc)
                        z_src = zb
                    else:
                        z_src = z_ps

                    mvs1 = msc.tile([128, 4, 2], F32, tag="mvs1", name=f"mv1{nb}")
                    bst1 = msc.tile([128, 4, 6], F32, tag="bst1", name=f"b1s{nb}")
                    for pos in range(4):
                        nc.vector.bn_stats(out=bst1[:, pos, :], in_=z_src[:, pos, :])
                        nc.vector.bn_aggr(out=mvs1[:, pos, :], in_=bst1[:, pos, :])
                    rs1 = newton_rsqrt(nc.vector, nc.gpsimd, mvs1[:, :, 1], 4,
                                       "1", msc)
                    nb1 = msc.tile([128, 4], F32, tag="nb1", name=f"nb1{nb}")
                    # nb1 = -mean1 * rs1
                    nc.vector.scalar_tensor_tensor(
                        out=nb1, in0=mvs1[:, :, 0], scalar=-1.0, in1=rs1,
                        op0=OP.mult, op1=OP.mult)

                    y2 = sb.tile([128, 4, 256], F32, tag="y2", name=f"y2{nb}")
                    if use_g1:
                        # ln1 = z*rs1 + nb1 on ACT; *g1 + be1; y2 = ln1 + ys
                        ln1 = sb.tile([128, 4, 256], BF16, tag="ln1", name=f"ln1{nb}")
                        for pos, (ct, j) in enumerate(chunks):
                            nc.scalar.activation(out=ln1[:, pos, :],
                                                 in_=z_src[:, pos, :],
                                                 func=AF.Identity,
                                                 scale=rs1[:, pos:pos + 1],
                                                 bias=nb1[:, pos:pos + 1])
                            nc.vector.tensor_mul(ln1[:, pos, :], ln1[:, pos, :], g1_bc)
                            nc.vector.tensor_add(ln1[:, pos, :], ln1[:, pos, :], be1_bc)
                            nc.gpsimd.tensor_add(
                                y2[:, pos, :], ln1[:, pos, :],
                                ys_all[:, ct, hn, j * 256:(j + 1) * 256])
                    else:
                        # chunks 0,1: fused y2 = (z*rs1 + nb1) + ys on DVE;
                        # chunks 2,3: ln1 on ACT + add on Pool (engine balance)
                        from concourse.dve_ops import AFFINE_THEN_ADD
                        ln1 = sb.tile([128, 2, 256], BF16, tag="ln1", name=f"ln1{nb}")
                        for pos, (ct, j) in enumerate(chunks):
                            if pos < 2:
                                nc.vector._custom_dve(
                                    AFFINE_THEN_ADD,
                                    out=y2[:, pos, :], in0=z_src[:, pos, :],
                                    in1=ys_all[:, ct, hn, j * 256:(j + 1) * 256],
                                    s0=rs1[:, pos:pos + 1], s1=nb1[:, pos:pos + 1])
                            else:
                                nc.scalar.activation(out=ln1[:, pos - 2, :],
                                                     in_=z_src[:, pos, :],
                                                     func=AF.Identity,
                                                     scale=rs1[:, pos:pos + 1],
                                                     bias=nb1[:, pos:pos + 1])
                                nc.gpsimd.tensor_add(
                                    y2[:, pos, :], ln1[:, pos - 2, :],
                                    ys_all[:, ct, hn, j * 256:(j + 1) * 256])

                    mvs2 = msc.tile([128, 4, 2], F32, tag="mvs2", name=f"mv2{nb}")
                    bst2 = msc.tile([128, 4, 6], F32, tag="bst2", name=f"b2s{nb}")
                    for pos in range(4):
                        nc.vector.bn_stats(out=bst2[:, pos, :], in_=y2[:, pos, :])
                        nc.vector.bn_aggr(out=mvs2[:, pos, :], in_=bst2[:, pos, :])
                    rs2 = newton_rsqrt(nc.vector, nc.gpsimd, mvs2[:, :, 1], 4,
                                       "2", msc)

                    outt = sb.tile([128, 4, 256], F32, tag="outt", name=f"out{nb}")
                    if use_g2:
                        nb2 = msc.tile([128, 4], F32, tag="nb2", name=f"nb2{nb}")
                        nc.vector.scalar_tensor_tensor(
                            out=nb2, in0=mvs2[:, :, 0], scalar=-1.0, in1=rs2,
                            op0=OP.mult, op1=OP.mult)
                        ln2 = sb.tile([128, 4, 256], F32, tag="ln2", name=f"ln2{nb}")
                        for pos in range(4):
                            nc.vector.tensor_scalar(
                                out=ln2[:, pos, :], in0=y2[:, pos, :],
                                scalar1=rs2[:, pos:pos + 1],
                                scalar2=nb2[:, pos:pos + 1],
                                op0=OP.mult, op1=OP.add)
                            nc.vector.tensor_mul(ln2[:, pos, :], ln2[:, pos, :], g2_bc)
                            nc.vector.tensor_add(ln2[:, pos, :], ln2[:, pos, :], be2_bc)
                            nc.gpsimd.tensor_add(outt[:, pos, :], ln2[:, pos, :],
                                                 y2[:, pos, :])
                    else:
                        # outt = y2*(1+rs2) - mean2*rs2, fused
                        sc1 = msc.tile([128, 4], F32, tag="sc1", name=f"sc1{nb}")
                        nc.vector.tensor_scalar(out=sc1, in0=rs2, scalar1=1.0,
                                                scalar2=None, op0=OP.add)
                        sc2 = msc.tile([128, 4], F32, tag="sc2", name=f"sc2{nb}")
                        nc.vector.scalar_tensor_tensor(
                            out=sc2, in0=mvs2[:, :, 0], scalar=-1.0, in1=rs2,
                            op0=OP.mult, op1=OP.mult)
                        for pos in range(4):
                            nc.gpsimd.tensor_scalar(
                                out=outt[:, pos, :], in0=y2[:, pos, :],
                                scalar1=sc1[:, pos:pos + 1],
                                scalar2=sc2[:, pos:pos + 1],
                                op0=OP.mult, op1=OP.add)
                    for pos, (ct, j) in enumerate(chunks):
                        nc.sync.dma_start(
                            out=OUTV[ct * 128:(ct + 1) * 128, hn * 8 + j, :],
                            in_=outt[:, pos, :])
                    fst[nb] = {}

                for slot in range(NNB + 2):
                    if slot < NNB:
                        stage_yt(slot)
                    if 1 <= slot <= NNB:
                        stage_h(slot - 1)
                    if 2 <= slot:
                        stage_f(slot - 2)
                        stage_ln(slot - 2)

    nc.compile()
    return nc


def _get_program(flags):
    if flags not in _CACHE:
        _CACHE[flags] = _build(flags)
    return _CACHE[flags]


def kernel(**inputs):
    import ml_dtypes
    BF = ml_dtypes.bfloat16

    x = np.asarray(inputs["x"], np.float32)
    Wq = np.asarray(inputs["Wq"], np.float32)
    Wk = np.asarray(inputs["Wk"], np.float32)
    Wv = np.asarray(inputs["Wv"], np.float32)
    Wo = np.asarray(inputs["Wo"], np.float32)
    W1 = np.asarray(inputs["W1"], np.float32)
    W2 = np.asarray(inputs["W2"], np.float32)
    bq = np.asarray(inputs["bq"], np.float32)
    bk = np.asarray(inputs["bk"], np.float32)
    bv = np.asarray(inputs["bv"], np.float32)
    bo = np.asarray(inputs["bo"], np.float32)
    b1 = np.asarray(inputs["b1"], np.float32)
    b2 = np.asarray(inputs["b2"], np.float32)
    g1 = np.asarray(inputs["g1"], np.float32)
    be1 = np.asarray(inputs["be1"], np.float32)
    g2 = np.asarray(inputs["g2"], np.float32)
    be2 = np.asarray(inputs["be2"], np.float32)

    flags = (
        bool(bq.any() or bk.any()),
        bool(bv.any()),
        bool(bo.any()),
        bool(b1.any()),
        bool(b2.any()),
        bool((g1 != 1.0).any() or be1.any()),
        bool((g2 != 1.0).any() or be2.any()),
    )
    nc = _get_program(flags)

    scale = 1.0 / np.sqrt(np.float32(E))
    base = {
        "wq": (Wq * scale).astype(BF),
        "wk": Wk.astype(BF),
        "wv": Wv.astype(BF),
        "wo": Wo.astype(BF),
        "w1": W1.astype(BF),
        "w2": W2.astype(BF),
        "ident": np.eye(128, dtype=np.float32).astype(BF),
    }
    use_bqk, use_bv, use_bo, use_b1, use_b2, use_g1, use_g2 = flags
    if use_bqk:
        base["bq"] = bq * scale
        base["bk"] = bk
    if use_bv:
        base["bv"] = bv
    if use_bo:
        base["bo"] = bo
    if use_b1:
        base["b1"] = b1
    if use_b2:
        base["b2"] = b2
    if use_g1:
        base["g1"] = g1
        base["be1"] = be1
    if use_g2:
        base["g2"] = g2
        base["be2"] = be2

    in_maps = [dict(base, x=x[b].astype(BF)) for b in range(B)]

    from concourse.bass_utils import run_bass_kernel_spmd

    trace = os.environ.get("TRN_TRACE") == "1"
    tmpdir = os.environ.get("TRN_TRACE_DIR") or None
    res = run_bass_kernel_spmd(nc, in_maps, list(range(B)), trace=trace,
                               tmpdir=tmpdir)
    kernel.last_exec_time_ns = res.exec_time_ns
    kernel.last_profile_json = res.profile_json
    kernel.last_trace = res.instructions_and_trace
    return np.stack([r["out"] for r in res.results], axis=0)


# revision 42
# speedup vs baseline: 1.2095x; 1.0996x over previous
"""GridTransformerBlock TRN2 kernel (v2).

Sharding: batch-parallel over B=8 -> one batch per NeuronCore, zero
collectives.

v2 design (vs v1 baseline at 1.147ms):
- bf16 operands everywhere (weights + activations). At N>=256 the PE streams
  1 row/cycle for both f32r and bf16, but bf16 gets Fast Weight Load
  (f32r LDWEIGHTS was 226ns x 3584 = 810us of PE weight-path time), 1.0
  (vs 1.5) cycles/row transposes, and halves SBUF/DMA traffic.
- Phase split: ALL window attention first (ACT table = exp only), then ALL
  FFN (ACT table = gelu only). v1 alternated per stripe and paid ~44
  ACT_TABLE_LOADs. ys (8.4MB bf16) stays SBUF-resident between phases.
- Window-pair batching: Q/K projections stream N=512 through one weight
  load; software-pipelined slot schedule keeps the PE fed across the
  transpose->copy->matmul dependency chains (v1 had 590us of HAM throttle
  from PE idle gaps).
- FFN W2 matmul emits token-major output directly (lhsT = gelu chunks),
  killing the f^T transposes + copies; LN epilogue fused into
  tensor_scalar ops; 2-iteration Newton rsqrt.
"""

import os
import sys
import numpy as np

for _p in ("/opt/trn_rl_repo", "/root/.axon_site/_ro/trn_rl_repo"):
    if _p not in sys.path and os.path.isdir(_p):
        sys.path.insert(0, _p)

B, S, E, FF = 8, 16384, 256, 1024
H, W, G = 128, 128, 16
Hn, Wn = 8, 8
NWP = Hn * (Wn // 2)   # 32 window-pairs
NNB = 32               # 32 FFN blocks of 512 tokens

_CACHE = {}


def _build(flags):
    use_bqk, use_bv, use_bo, use_b1, use_b2, use_g1, use_g2 = flags
    import concourse.bacc as bacc
    import concourse.mybir as mybir
    import concourse.tile as tile
    from contextlib import ExitStack

    F32 = mybir.dt.float32
    BF16 = mybir.dt.bfloat16
    I32 = mybir.dt.int32
    AF = mybir.ActivationFunctionType
    OP = mybir.AluOpType

    nc = bacc.Bacc("TRN2", target_bir_lowering=False, debug=False, num_devices=8)

    x_d = nc.dram_tensor("x", [S, E], BF16, kind="ExternalInput")
    wq_d = nc.dram_tensor("wq", [E, E], BF16, kind="ExternalInput")
    wk_d = nc.dram_tensor("wk", [E, E], BF16, kind="ExternalInput")
    wv_d = nc.dram_tensor("wv", [E, E], BF16, kind="ExternalInput")
    wo_d = nc.dram_tensor("wo", [E, E], BF16, kind="ExternalInput")
    w1_d = nc.dram_tensor("w1", [E, FF], BF16, kind="ExternalInput")
    w2_d = nc.dram_tensor("w2", [FF, E], BF16, kind="ExternalInput")
    id_d = nc.dram_tensor("ident", [128, 128], BF16, kind="ExternalInput")
    out_d = nc.dram_tensor("out", [S, E], F32, kind="ExternalOutput")
    if use_bqk:
        bq_d = nc.dram_tensor("bq", [E], F32, kind="ExternalInput")
        bk_d = nc.dram_tensor("bk", [E], F32, kind="ExternalInput")
    if use_bv:
        bv_d = nc.dram_tensor("bv", [E], F32, kind="ExternalInput")
    if use_bo:
        bo_d = nc.dram_tensor("bo", [E], F32, kind="ExternalInput")
    if use_b1:
        b1_d = nc.dram_tensor("b1", [FF], F32, kind="ExternalInput")
    if use_b2:
        b2_d = nc.dram_tensor("b2", [E], F32, kind="ExternalInput")
    if use_g1:
        g1_d = nc.dram_tensor("g1", [E], F32, kind="ExternalInput")
        be1_d = nc.dram_tensor("be1", [E], F32, kind="ExternalInput")
    if use_g2:
        g2_d = nc.dram_tensor("g2", [E], F32, kind="ExternalInput")
        be2_d = nc.dram_tensor("be2", [E], F32, kind="ExternalInput")

    import concourse.bass as bass

    def bcast_ap(dram, n=256):
        return bass.AP(tensor=dram.ap().tensor, offset=0, ap=[[0, 128], [1, n]])

    X = x_d.ap().rearrange("(c t) e -> c (t e)", t=64)      # [256, 16384] bf16
    OUTV = out_d.ap().rearrange("(c t) e -> c t e", t=64)   # [256, 64, 256] f32

    with tile.TileContext(nc) as tc:
        with ExitStack() as ctx:
            const = ctx.enter_context(tc.tile_pool(name="const", bufs=1))

            ident = const.tile([128, 128], BF16)
            nc.sync.dma_start(out=ident, in_=id_d.ap()[:, :])
            wq_t = const.tile([128, 2, 256], BF16)
            wk_t = const.tile([128, 2, 256], BF16)
            wv_t = const.tile([128, 2, 256], BF16)
            wo_t = const.tile([128, 2, 256], BF16)
            for t, d in ((wq_t, wq_d), (wk_t, wk_d), (wv_t, wv_d), (wo_t, wo_d)):
                nc.sync.dma_start(out=t, in_=d.ap().rearrange("(eh k) f -> k eh f", k=128))
            w1_t = const.tile([128, 2, 1024], BF16)
            nc.sync.dma_start(out=w1_t, in_=w1_d.ap().rearrange("(eh k) f -> k eh f", k=128))
            w2_t = const.tile([128, 8, 256], BF16)
            nc.sync.dma_start(out=w2_t, in_=w2_d.ap().rearrange("(fm k) e -> k fm e", k=128))

            if use_bqk:
                bq_t = const.tile([128, 2], F32)
                nc.sync.dma_start(out=bq_t, in_=bq_d.ap().rearrange("(fh p) -> p fh", p=128))
                bk_t = const.tile([128, 2], F32)
                nc.sync.dma_start(out=bk_t, in_=bk_d.ap().rearrange("(fh p) -> p fh", p=128))
            if use_bv:
                bv_bc = const.tile([128, 256], F32)
                nc.sync.dma_start(out=bv_bc, in_=bcast_ap(bv_d))
            if use_bo:
                bo_st = const.tile([128, 2048], F32)
                nc.sync.dma_start(
                    out=bo_st.rearrange("p (g1 wn g2) -> p g1 wn g2", wn=8, g2=16),
                    in_=bass.AP(tensor=bo_d.ap().tensor, offset=0,
                                ap=[[0, 128], [16, 16], [0, 8], [1, 16]]))
            if use_b1:
                b1_t = const.tile([128, 8], F32)
                nc.sync.dma_start(out=b1_t, in_=b1_d.ap().rearrange("(fm p) -> p fm", p=128))
            if use_b2:
                b2_bc = const.tile([128, 256], F32)
                nc.sync.dma_start(out=b2_bc, in_=bcast_ap(b2_d))
            if use_g1:
                g1_bc = const.tile([128, 256], F32)
                nc.sync.dma_start(out=g1_bc, in_=bcast_ap(g1_d))
                be1_bc = const.tile([128, 256], F32)
                nc.sync.dma_start(out=be1_bc, in_=bcast_ap(be1_d))
            if use_g2:
                g2_bc = const.tile([128, 256], F32)
                nc.sync.dma_start(out=g2_bc, in_=bcast_ap(g2_d))
                be2_bc = const.tile([128, 256], F32)
                nc.sync.dma_start(out=be2_bc, in_=bcast_ap(be2_d))

            # attention output, SBUF-resident across phases:
            # [c-part, ct(channel half), hn(stripe), 2048 pixels(g1, w)]
            ys_all = const.tile([128, 2, 8, 2048], BF16)

            def newton_rsqrt(eng_seed, eng_iter, var_ap, n, tagp, pool, iters=1):
                """rstd = 1/sqrt(var + eps) for a [128, n] strided var AP.
                Bit-trick seed on eng_seed (must be DVE); NR iteration
                (mult/add only) can run on Pool."""
                w = pool.tile([128, n], F32, tag=f"nw_w{tagp}", name=f"nw_w{tagp}")
                eng_seed.tensor_scalar(out=w, in0=var_ap, scalar1=1e-5,
                                       scalar2=None, op0=OP.add)
                r = pool.tile([128, n], F32, tag=f"nw_r{tagp}", name=f"nw_r{tagp}")
                eng_seed.tensor_scalar(out=r.bitcast(I32), in0=w.bitcast(I32),
                                       scalar1=1, scalar2=None,
                                       op0=OP.logical_shift_right)
                eng_seed.tensor_scalar(out=r.bitcast(I32), in0=r.bitcast(I32),
                                       scalar1=0xFFFFFFFF, scalar2=None,
                                       op0=OP.bitwise_xor)
                eng_seed.tensor_scalar(out=r.bitcast(I32), in0=r.bitcast(I32),
                                       scalar1=0x5F375A86 + 1, scalar2=None,
                                       op0=OP.add)
                rsq = pool.tile([128, n], F32, tag=f"nw_q{tagp}", name=f"nw_q{tagp}")
                u = pool.tile([128, n], F32, tag=f"nw_u{tagp}", name=f"nw_u{tagp}")
                v = pool.tile([128, n], F32, tag=f"nw_v{tagp}", name=f"nw_v{tagp}")
                for _ in range(iters):
                    eng_iter.tensor_mul(rsq, r, r)
                    eng_iter.tensor_mul(u, rsq, w)
                    eng_iter.tensor_scalar(out=v, in0=u, scalar1=-0.5, scalar2=1.5,
                                           op0=OP.mult, op1=OP.add)
                    eng_iter.tensor_mul(r, r, v)
                return r

            # ================= Phase A: window attention =================
            with ExitStack() as pa:
                xsp = pa.enter_context(tc.tile_pool(name="xsp", bufs=2))
                sa = pa.enter_context(tc.tile_pool(name="sa", bufs=2))
                stp = pa.enter_context(tc.tile_pool(name="stp", bufs=3))
                psR = pa.enter_context(tc.tile_pool(name="psR", bufs=3, space="PSUM"))
                psT = pa.enter_context(tc.tile_pool(name="psT", bufs=2, space="PSUM"))

                xs_tiles = {}

                def load_stripe(hn):
                    t = xsp.tile([128, 2, 2048], BF16, tag="xs", name=f"xs{hn}")
                    for ct in range(2):
                        nc.sync.dma_start(
                            out=t[:, ct, :],
                            in_=X[ct * 128:(ct + 1) * 128, hn * 2048:(hn + 1) * 2048])
                    xs_tiles[hn] = t

                # per-wp state kept across pipeline slots
                st = [dict() for _ in range(NWP)]

                load_stripe(0)
                load_stripe(1)

                def stage_gather(i):
                    """Pool-gather window-pair i's tokens into contiguous t_sb."""
                    hn, wp = divmod(i, 4)
                    if wp == 0 and hn + 2 < Hn:
                        load_stripe(hn + 2)
                    xs = xs_tiles[hn]
                    xv = xs.rearrange("p ct (g1 w) -> p ct g1 w", w=128)
                    t_sb = sa.tile([128, 2, 2, 16, 16], BF16, tag="tsb", name=f"tsb{i}")
                    for ct in range(2):
                        nc.gpsimd.tensor_copy(
                            t_sb[:, ct, :, :, :],
                            xv[:, ct, :, wp * 32:(wp + 1) * 32].rearrange(
                                "p g1 (w g2) -> p w g1 g2", g2=16))
                    st[i]["t_sb"] = t_sb

                def stage_tt(i):
                    """Transpose the 2 windows' tokens: tt = t^T [pix, (w c)]."""
                    t_sb = st[i]["t_sb"].rearrange("p ct w g1 g2 -> p ct (w g1 g2)")
                    tt_ps = psT.tile([128, 2, 2, 256], BF16, tag="t16", name=f"ttp{i}")
                    for eh in range(2):
                        for w in range(2):
                            for ct in range(2):
                                nc.tensor.transpose(
                                    tt_ps[:, eh, w, ct * 128:(ct + 1) * 128],
                                    t_sb[:, ct,
                                         w * 256 + eh * 128:w * 256 + (eh + 1) * 128],
                                    ident)
                    tt = sa.tile([128, 2, 2, 256], BF16, tag="tt", name=f"tt{i}")
                    nc.vector.tensor_copy(tt, tt_ps)
                    st[i]["tt"] = tt

                def stage_q(i):
                    tt = st[i]["tt"]
                    qt_ps = psR.tile([128, 2, 512], F32, tag="r", name=f"qtp{i}")
                    for fh in range(2):
                        for eh in range(2):
                            nc.tensor.matmul(qt_ps[:, fh, :],
                                             lhsT=wq_t[:, eh, fh * 128:(fh + 1) * 128],
                                             rhs=tt[:, eh, :, :],
                                             start=eh == 0, stop=eh == 1)
                    qt = sa.tile([128, 2, 2, 256], BF16, tag="qt", name=f"qt{i}")
                    qv = qt.rearrange("p fh w c -> p fh (w c)")
                    if use_bqk:
                        for fh in range(2):
                            nc.scalar.activation(out=qv[:, fh, :], in_=qt_ps[:, fh, :],
                                                 func=AF.Identity,
                                                 bias=bq_t[:, fh:fh + 1])
                    else:
                        nc.scalar.activation(out=qv, in_=qt_ps, func=AF.Copy)
                    st[i]["qt"] = qt

                def stage_k(i):
                    tt = st[i]["tt"]
                    kt_ps = psR.tile([128, 2, 512], F32, tag="r", name=f"ktp{i}")
                    for fh in range(2):
                        for eh in range(2):
                            nc.tensor.matmul(kt_ps[:, fh, :],
                                             lhsT=wk_t[:, eh, fh * 128:(fh + 1) * 128],
                                             rhs=tt[:, eh, :, :],
                                             start=eh == 0, stop=eh == 1)
                    kt = sa.tile([128, 2, 2, 256], BF16, tag="kt", name=f"kt{i}")
                    kv = kt.rearrange("p fh w c -> p fh (w c)")
                    if use_bqk:
                        for fh in range(2):
                            nc.vector.tensor_scalar(
                                out=kv[:, fh, :], in0=kt_ps[:, fh, :],
                                scalar1=bk_t[:, fh:fh + 1], scalar2=None,
                                op0=OP.add)
                    else:
                        nc.vector.tensor_copy(kv, kt_ps)
                    st[i]["kt"] = kt

                def stage_v(i):
                    tt = st[i]["tt"]
                    v_ps = psR.tile([128, 2, 2, 256], F32, tag="r", name=f"vp{i}")
                    for w in range(2):
                        for ch in range(2):
                            for eh in range(2):
                                nc.tensor.matmul(
                                    v_ps[:, w, ch, :],
                                    lhsT=tt[:, eh, w, ch * 128:(ch + 1) * 128],
                                    rhs=wv_t[:, eh, :],
                                    start=eh == 0, stop=eh == 1)
                    vv = sa.tile([128, 2, 2, 256], BF16, tag="vv", name=f"vv{i}",
                                 bufs=3)
                    if use_bv:
                        for w in range(2):
                            for ch in range(2):
                                nc.vector.tensor_add(vv[:, w, ch, :],
                                                     v_ps[:, w, ch, :], bv_bc)
                    else:
                        nc.scalar.activation(out=vv, in_=v_ps, func=AF.Copy)
                    st[i]["vv"] = vv

                def stage_s(i):
                    qt, kt = st[i]["qt"], st[i]["kt"]
                    s_ps = psR.tile([128, 2, 2, 256], F32, tag="r", name=f"sp{i}")
                    for th in range(2):
                        for w in range(2):
                            for fh in range(2):
                                nc.tensor.matmul(
                                    s_ps[:, th, w, :],
                                    lhsT=qt[:, fh, w, th * 128:(th + 1) * 128],
                                    rhs=kt[:, fh, w, :],
                                    start=fh == 0, stop=fh == 1)
                    st[i]["s_ps"] = s_ps

                def stage_exp(i):
                    s_ps = st[i]["s_ps"]
                    aa = sa.tile([128, 2, 2, 256], BF16, tag="aa", name=f"aa{i}")
                    den = stp.tile([128, 2, 2], F32, tag="den", name=f"den{i}")
                    for th in range(2):
                        for w in range(2):
                            nc.scalar.activation(out=aa[:, th, w, :],
                                                 in_=s_ps[:, th, w, :],
                                                 func=AF.Exp,
                                                 accum_out=den[:, th, w:w + 1])
                    st[i]["aa"] = aa
                    st[i]["den"] = den

                def stage_at(i):
                    aa = st[i]["aa"]
                    at_ps = psT.tile([128, 2, 2, 256], BF16, tag="t16", name=f"atp{i}")
                    for t2h in range(2):
                        for w in range(2):
                            for th in range(2):
                                nc.tensor.transpose(
                                    at_ps[:, t2h, w, th * 128:(th + 1) * 128],
                                    aa[:, th, w, t2h * 128:(t2h + 1) * 128],
                                    ident)
                    at = sa.tile([128, 2, 2, 256], BF16, tag="at", name=f"at{i}")
                    nc.vector.tensor_copy(at, at_ps)
                    st[i]["at"] = at

                def stage_ot(i):
                    vv, at = st[i]["vv"], st[i]["at"]
                    ot_ps = psR.tile([128, 2, 2, 256], F32, tag="r", name=f"otp{i}")
                    for w in range(2):
                        for fh in range(2):
                            for t2h in range(2):
                                nc.tensor.matmul(
                                    ot_ps[:, w, fh, :],
                                    lhsT=vv[:, w, t2h, fh * 128:(fh + 1) * 128],
                                    rhs=at[:, t2h, w, :],
                                    start=t2h == 0, stop=t2h == 1)
                    ot = sa.tile([128, 2, 2, 256], BF16, tag="ot", name=f"ot{i}")
                    nc.vector.tensor_copy(ot, ot_ps)
                    st[i]["ot"] = ot

                def stage_o2(i):
                    hn, wp = divmod(i, 4)
                    ot, den = st[i]["ot"], st[i]["den"]
                    o2_ps = psR.tile([128, 2, 2, 256], F32, tag="r", name=f"o2p{i}")
                    for w in range(2):
                        for th in range(2):
                            for fh in range(2):
                                nc.tensor.matmul(
                                    o2_ps[:, w, th, :],
                                    lhsT=ot[:, w, fh, th * 128:(th + 1) * 128],
                                    rhs=wo_t[:, fh, :],
                                    start=fh == 0, stop=fh == 1)
                    rec = stp.tile([128, 2, 2], F32, tag="rec", name=f"rec{i}")
                    nc.vector.reciprocal(rec, den)
                    ysv = ys_all.rearrange("p ct hn (g1 w) -> p ct hn g1 w", w=128)
                    for w in range(2):
                        wn = wp * 2 + w
                        for th in range(2):
                            nc.vector.tensor_scalar(
                                out=ysv[:, th, hn, :, wn * 16:(wn + 1) * 16],
                                in0=o2_ps[:, w, th, :].rearrange(
                                    "p (a b) -> p a b", b=16),
                                scalar1=rec[:, th, w:w + 1], scalar2=None,
                                op0=OP.mult)
                    st[i] = {}

                # software-pipelined slot schedule (3-deep: at/ot one slot
                # after exp, o2 one more, so each PSUM->SBUF copy has a full
                # slot of slack before its consumer)
                stage_gather(0)
                for slot in range(NWP + 3):
                    if slot < NWP:
                        stage_tt(slot)
                    if slot + 1 < NWP:
                        stage_gather(slot + 1)
                    if 2 <= slot <= NWP + 1:
                        stage_at(slot - 2)
                    if 1 <= slot <= NWP:
                        stage_s(slot - 1)
                        stage_exp(slot - 1)
                    if 3 <= slot:
                        stage_o2(slot - 3)
                    if slot < NWP:
                        stage_q(slot)
                        stage_k(slot)
                        stage_v(slot)
                    if 2 <= slot <= NWP + 1:
                        stage_ot(slot - 2)

                if use_bo:
                    for ct in range(2):
                        for hn in range(Hn):
                            nc.gpsimd.tensor_add(
                                ys_all[:, ct, hn, :],
                                ys_all[:, ct, hn, :].bitcast(BF16), bo_st)

            # ================= Phase B: FFN + 2 post-LNs =================
            with ExitStack() as pb:
                sb = pb.enter_context(tc.tile_pool(name="sb", bufs=2))
                msc = pb.enter_context(tc.tile_pool(name="msc", bufs=2))
                psH = pb.enter_context(tc.tile_pool(name="psH", bufs=3, space="PSUM"))
                psZ = pb.enter_context(tc.tile_pool(name="psZ", bufs=2, space="PSUM"))
                psY = pb.enter_context(tc.tile_pool(name="psY", bufs=1, space="PSUM"))

                fst = [dict() for _ in range(NNB)]

                def chunks_of(nb):
                    # 4 token chunks (ct, j) within stripe hn
                    return nb // 4, [(q // 8, q % 8) for q in
                                     range((nb % 4) * 4, (nb % 4) * 4 + 4)]

                def stage_yt(nb):
                    """Transpose 4 token chunks into feature-major yt."""
                    hn, chunks = chunks_of(nb)
                    yt_ps = psY.tile([128, 2, 512], BF16, tag="yt16", name=f"ytp{nb}")
                    for eh in range(2):
                        for pos, (ct, j) in enumerate(chunks):
                            nc.tensor.transpose(
                                yt_ps[:, eh, pos * 128:(pos + 1) * 128],
                                ys_all[:, ct, hn,
                                       j * 256 + eh * 128:j * 256 + (eh + 1) * 128],
                                ident)
                    yt = sb.tile([128, 2, 512], BF16, tag="yt", name=f"yt{nb}")
                    nc.scalar.activation(out=yt, in_=yt_ps, func=AF.Copy)
                    fst[nb]["yt"] = yt

                def stage_h(nb):
                    yt = fst[nb]["yt"]
                    hh = sb.tile([128, 8, 512], BF16, tag="hh", name=f"hh{nb}")
                    for fm in range(8):
                        h_ps = psH.tile([128, 512], F32, tag="h", name=f"hp{nb}_{fm}")
                        for eh in range(2):
                            nc.tensor.matmul(
                                h_ps,
                                lhsT=w1_t[:, eh, fm * 128:(fm + 1) * 128],
                                rhs=yt[:, eh, :],
                                start=eh == 0, stop=eh == 1)
                        nc.scalar.activation(out=hh[:, fm, :], in_=h_ps,
                                             func=AF.Gelu,
                                             bias=b1_t[:, fm:fm + 1] if use_b1
                                             else 0.0)
                    fst[nb]["hh"] = hh

                def stage_f(nb):
                    hh = fst[nb]["hh"]
                    z_ps = psZ.tile([128, 4, 256], F32, tag="z", name=f"zp{nb}")
                    for pos in range(4):
                        for fm in range(8):
                            nc.tensor.matmul(
                                z_ps[:, pos, :],
                                lhsT=hh[:, fm, pos * 128:(pos + 1) * 128],
                                rhs=w2_t[:, fm, :],
                                start=fm == 0, stop=fm == 7)
                    fst[nb]["z_ps"] = z_ps

                def stage_ln(nb):
                    hn, chunks = chunks_of(nb)
                    z_ps = fst[nb]["z_ps"]
                    if use_b2:
                        zb = sb.tile([128, 4, 256], F32, tag="zb", name=f"zb{nb}")
                        for pos in range(4):
                            nc.vector.tensor_add(zb[:, pos, :], z_ps[:, pos, :], b2_bc)
                        z_src = zb
                    else:
                        z_src = z_ps

                    mvs1 = msc.tile([128, 4, 2], F32, tag="mvs1", name=f"mv1{nb}")
                    bst1 = msc.tile([128, 4, 6], F32, tag="bst1", name=f"b1s{nb}")
                    for pos in range(4):
                        nc.vector.bn_stats(out=bst1[:, pos, :], in_=z_src[:, pos, :])
                        nc.vector.bn_aggr(out=mvs1[:, pos, :], in_=bst1[:, pos, :])
                    rs1 = newton_rsqrt(nc.vector, nc.gpsimd, mvs1[:, :, 1], 4,
                                       "1", msc)
                    nb1 = msc.tile([128, 4], F32, tag="nb1", name=f"nb1{nb}")
                    # nb1 = -mean1 * rs1
                    nc.vector.scalar_tensor_tensor(
                        out=nb1, in0=mvs1[:, :, 0], scalar=-1.0, in1=rs1,
                        op0=OP.mult, op1=OP.mult)

                    y2 = sb.tile([128, 4, 256], F32, tag="y2", name=f"y2{nb}")
                    # ln1 = z*rs1 + nb1 on DVE (reads z from PSUM, frees it
                    # early); y2 = ln1 + ys residual adds on Pool
                    ln1 = sb.tile([128, 4, 256], BF16, tag="ln1", name=f"ln1{nb}")
                    for pos, (ct, j) in enumerate(chunks):
                        nc.vector.tensor_scalar(
                            out=ln1[:, pos, :], in0=z_src[:, pos, :],
                            scalar1=rs1[:, pos:pos + 1],
                            scalar2=nb1[:, pos:pos + 1],
                            op0=OP.mult, op1=OP.add)
                        if use_g1:
                            nc.vector.tensor_mul(ln1[:, pos, :], ln1[:, pos, :], g1_bc)
                            nc.vector.tensor_add(ln1[:, pos, :], ln1[:, pos, :], be1_bc)
                        nc.gpsimd.tensor_add(
                            y2[:, pos, :], ln1[:, pos, :],
                            ys_all[:, ct, hn, j * 256:(j + 1) * 256])

                    mvs2 = msc.tile([128, 4, 2], F32, tag="mvs2", name=f"mv2{nb}")
                    bst2 = msc.tile([128, 4, 6], F32, tag="bst2", name=f"b2s{nb}")
                    for pos in range(4):
                        nc.vector.bn_stats(out=bst2[:, pos, :], in_=y2[:, pos, :])
                        nc.vector.bn_aggr(out=mvs2[:, pos, :], in_=bst2[:, pos, :])
                    rs2 = newton_rsqrt(nc.vector, nc.gpsimd, mvs2[:, :, 1], 4,
                                       "2", msc)

                    outt = sb.tile([128, 4, 256], F32, tag="outt", name=f"out{nb}")
                    if use_g2:
                        nb2 = msc.tile([128, 4], F32, tag="nb2", name=f"nb2{nb}")
                        nc.vector.scalar_tensor_tensor(
                            out=nb2, in0=mvs2[:, :, 0], scalar=-1.0, in1=rs2,
                            op0=OP.mult, op1=OP.mult)
                        ln2 = sb.tile([128, 4, 256], F32, tag="ln2", name=f"ln2{nb}")
                        for pos in range(4):
                            nc.vector.tensor_scalar(
                                out=ln2[:, pos, :], in0=y2[:, pos, :],
                                scalar1=rs2[:, pos:pos + 1],
                                scalar2=nb2[:, pos:pos + 1],
                                op0=OP.mult, op1=OP.add)
                            nc.vector.tensor_mul(ln2[:, pos, :], ln2[:, pos, :], g2_bc)
                            nc.vector.tensor_add(ln2[:, pos, :], ln2[:, pos, :], be2_bc)
                            nc.gpsimd.tensor_add(outt[:, pos, :], ln2[:, pos, :],
                                                 y2[:, pos, :])
                    else:
                        # outt = y2*(1+rs2) - mean2*rs2, fused
                        sc1 = msc.tile([128, 4], F32, tag="sc1", name=f"sc1{nb}")
                        nc.vector.tensor_scalar(out=sc1, in0=rs2, scalar1=1.0,
                                                scalar2=None, op0=OP.add)
                        sc2 = msc.tile([128, 4], F32, tag="sc2", name=f"sc2{nb}")
                        nc.vector.scalar_tensor_tensor(
                            out=sc2, in0=mvs2[:, :, 0], scalar=-1.0, in1=rs2,
                            op0=OP.mult, op1=OP.mult)
                        for pos in range(4):
                            nc.gpsimd.tensor_scalar(
                                out=outt[:, pos, :], in0=y2[:, pos, :],
                                scalar1=sc1[:, pos:pos + 1],
                                scalar2=sc2[:, pos:pos + 1],
                                op0=OP.mult, op1=OP.add)
                    for pos, (ct, j) in enumerate(chunks):
                        nc.sync.dma_start(
                            out=OUTV[ct * 128:(ct + 1) * 128, hn * 8 + j, :],
                            in_=outt[:, pos, :])
                    fst[nb] = {}

                for slot in range(NNB + 2):
                    if slot < NNB:
                        stage_yt(slot)
                    if 1 <= slot <= NNB:
                        stage_h(slot - 1)
                    if 2 <= slot:
                        stage_f(slot - 2)
                        stage_ln(slot - 2)

    nc.compile()
    return nc


def _get_program(flags):
    if flags not in _CACHE:
        _CACHE[flags] = _build(flags)
    return _CACHE[flags]


def kernel(**inputs):
    import ml_dtypes
    BF = ml_dtypes.bfloat16

    x = np.asarray(inputs["x"], np.float32)
    Wq = np.asarray(inputs["Wq"], np.float32)
    Wk = np.asarray(inputs["Wk"], np.float32)
    Wv = np.asarray(inputs["Wv"], np.float32)
    Wo = np.asarray(inputs["Wo"], np.float32)
    W1 = np.asarray(inputs["W1"], np.float32)
    W2 = np.asarray(inputs["W2"], np.float32)
    bq = np.asarray(inputs["bq"], np.float32)
    bk = np.asarray(inputs["bk"], np.float32)
    bv = np.asarray(inputs["bv"], np.float32)
    bo = np.asarray(inputs["bo"], np.float32)
    b1 = np.asarray(inputs["b1"], np.float32)
    b2 = np.asarray(inputs["b2"], np.float32)
    g1 = np.asarray(inputs["g1"], np.float32)
    be1 = np.asarray(inputs["be1"], np.float32)
    g2 = np.asarray(inputs["g2"], np.float32)
    be2 = np.asarray(inputs["be2"], np.float32)

    flags = (
        bool(bq.any() or bk.any()),
        bool(bv.any()),
        bool(bo.any()),
        bool(b1.any()),
        bool(b2.any()),
        bool((g1 != 1.0).any() or be1.any()),
        bool((g2 != 1.0).any() or be2.any()),
    )
    nc = _get_program(flags)

    scale = 1.0 / np.sqrt(np.float32(E))
    base = {
        "wq": (Wq * scale).astype(BF),
        "wk": Wk.astype(BF),
        "wv": Wv.astype(BF),
        "wo": Wo.astype(BF),
        "w1": W1.astype(BF),
        "w2": W2.astype(BF),
        "ident": np.eye(128, dtype=np.float32).astype(BF),
    }
    use_bqk, use_bv, use_bo, use_b1, use_b2, use_g1, use_g2 = flags
    if use_bqk:
        base["bq"] = bq * scale
        base["bk"] = bk
    if use_bv:
        base["bv"] = bv
    if use_bo:
        base["bo"] = bo
    if use_b1:
        base["b1"] = b1
    if use_b2:
        base["b2"] = b2
    if use_g1:
        base["g1"] = g1
        base["be1"] = be1
    if use_g2:
        base["g2"] = g2
        base["be2"] = be2

    in_maps = [dict(base, x=x[b].astype(BF)) for b in range(B)]

    from concourse.bass_utils import run_bass_kernel_spmd

    trace = os.environ.get("TRN_TRACE") == "1"
    tmpdir = os.environ.get("TRN_TRACE_DIR") or None
    res = run_bass_kernel_spmd(nc, in_maps, list(range(B)), trace=trace,
                               tmpdir=tmpdir)
    kernel.last_exec_time_ns = res.exec_time_ns
    kernel.last_profile_json = res.profile_json
    kernel.last_trace = res.instructions_and_trace
    return np.stack([r["out"] for r in res.results], axis=0)
